# revision 18
# baseline (speedup 1.0000x reference)
"""Causal multi-head attention (B=4, T=2048, D=1024, H=16) on 8 TRN2 NeuronCores.

Sharding: core c -> batch b = c // 2, head-group g = c % 2 (8 heads each).
Host pre-packs x^T per batch and W_qkv/W_o/biases per head-group into
partition-major bf16 layouts (1/sqrt(dh) folded into W_q/b_q on host).

Engines execute their instruction streams in order, so the build explicitly
software-pipelines: attention chunks (hp, c) emit slot-by-slot (scores ->
exp -> mask -> AV with 1-slot skew) and projection / output-projection
"filler" groups are interleaved between slots, paced evenly per region:

  head:  Q/K j0,j4 (tc0) and V tiles 0-3 for q/k range [0,512)
  c=0:   remaining th0 j-groups
  c=1:   V tiles 4-7, th1 j-groups (tc0), output projection of chunk 0
  c=2:   V tiles 8-11, th1 j-groups (tc1), oproj chunk 1
  c=3:   V tiles 12-15, oproj chunk 2;  tail: oproj chunk 3

Attention per slot: S^T = K^T.T Q^T (two heads row-packed, one 2-bank psum
tile), exp on ScalarE -> P^T bf16 (a tunable subset of off-diagonal slots
instead computes exp on VectorE via the Schraudolph bit trick: bf16 bits =
round(s*128*log2e + 16256 - 5.5), one tensor_scalar into an int16 view),
causal via suffix-trimming + triangular mask multiply on GpSimd, then
o^T (+rowsum via the ones-augmented V) accumulates in psum.  At chunk end
o^T is copied out of psum immediately (frees the accumulator), the rowsum
pair round-trips through DRAM as [128,8] so the exact DVE reciprocal runs
wide, and a second round trip broadcasts it for the normalize multiply.

Host sums the two head-group partials per batch and adds b_o.
"""

import sys

sys.path.insert(0, "/opt/trn_rl_repo")

import numpy as np

import concourse.bass as bass
import concourse.mybir as mybir
from concourse.bass_utils import run_bass_kernel_spmd
from concourse.tile import TileContext

F32 = mybir.dt.float32
BF16 = mybir.dt.bfloat16
I16 = mybir.dt.int16
EXP = mybir.ActivationFunctionType.Exp
MULT = None  # set after mybir import below

B, T, D, H = 4, 2048, 1024, 16
DH = D // H          # 64
HPC = H // 2         # heads per core = 8
DPC = HPC * DH       # 512 projected dims per core
N_CORES = 8
QC = 512             # q-chunk width in phase 2
KT = 128             # k-tile width

# Schraudolph fast-exp constants (bf16 bits = round(s*A + B))
A_EXP = 128.0 * 1.4426950408889634
B_EXP = 127.0 * 128.0 - 5.5


def use_dve_exp(hp, c, t):
    """Offload this slot's exp to VectorE? Only off-diagonal (unmasked,
    untrimmed) slots in the late ACT-bound regions."""
    if t >= 4 * c:          # diagonal-block tile
        return False
    return (c == 3 and t < 8) or (c == 2 and t < 4)


def split_excess_waits(nc, cap=1):
    """walrus limits sync-wait slots per ISA instruction (1 for several
    structs).  Move excess waits onto InstEventSemaphore instructions
    inserted just before the offender on the same engine."""
    n_split = 0
    for f in nc.m.functions:
        for blk in f.blocks:
            insts = blk.instructions
            out = []
            changed = False
            for inst in insts:
                si = inst.sync_info
                waits = list(si.on_wait) if si is not None else []
                if len(waits) > cap:
                    for j, w in enumerate(waits[:-cap]):
                        ev = mybir.InstEventSemaphore(
                            name=f"{inst.name}-w{j}", ins=[], outs=[]
                        )
                        ev.engine = inst.engine
                        ev.sync_info = mybir.SyncInfo(on_wait=[w], on_update=[])
                        out.append(ev)
                        n_split += 1
                    inst.sync_info = mybir.SyncInfo(
                        on_wait=waits[-cap:], on_update=list(si.on_update)
                    )
                    changed = True
                out.append(inst)
            if changed:
                blk.instructions = out
    return n_split


INST_LABELS = {}


def build():
    nc = bass.Bass(target_bir_lowering=False)

    _label = ["init"]

    def set_label(s):
        _label[0] = s

    for eng in (nc.tensor, nc.vector, nc.scalar, nc.gpsimd, nc.sync):
        orig = eng.add_instruction

        def wrapped(inst, _orig=orig):
            r = _orig(inst)
            try:
                INST_LABELS[inst.name] = _label[0]
            except Exception:
                pass
            return r

        eng.add_instruction = wrapped

    xT_d = nc.dram_tensor("xT", [128, 8, T], BF16, kind="ExternalInput")
    wqk_d = nc.dram_tensor("wqk", [128, 8, 8, 128], BF16, kind="ExternalInput")
    wv_d = nc.dram_tensor("wv", [128, 8, DPC], BF16, kind="ExternalInput")
    wo_d = nc.dram_tensor("wo", [128, 4, D], BF16, kind="ExternalInput")
    bqk_d = nc.dram_tensor("bqk", [128, 8], F32, kind="ExternalInput")
    bv_d = nc.dram_tensor("bv", [1, DPC], F32, kind="ExternalInput")
    mask_d = nc.dram_tensor("trimask", [128, 256], BF16, kind="ExternalInput")
    out_d = nc.dram_tensor("out", [T, D], F32, kind="ExternalOutput")
    rsum_d = nc.dram_tensor("rsum", [16, 1024], F32)  # internal scratch
    rrec_d = nc.dram_tensor("rrec", [16, 1024], F32)  # internal scratch

    with TileContext(nc) as tc:
        with (
            tc.tile_pool(name="const", bufs=1) as constp,
            tc.tile_pool(name="wstream", bufs=4) as wp,
            tc.tile_pool(name="xt", bufs=2) as xtp,
            tc.tile_pool(name="qk", bufs=1) as qkp,
            tc.tile_pool(name="vaug", bufs=1) as vp,
            tc.tile_pool(name="onorm", bufs=1) as onp,
            tc.tile_pool(name="pt", bufs=6) as ptp,
            tc.tile_pool(name="ocp", bufs=6) as ocpp,
            tc.tile_pool(name="small", bufs=8) as smallp,
            tc.tile_pool(name="osb", bufs=3) as osbp,
            tc.tile_pool(name="ps", bufs=4, space="PSUM") as psp,
            tc.tile_pool(name="spair", bufs=2, space="PSUM") as spp,
        ):
            # ---- persistent tiles ----
            qk_sb = [
                qkp.tile([128, T], BF16, tag=f"qk{j}", name=f"qk{j}")
                for j in range(8)
            ]
            vaug = [
                vp.tile([128, HPC, DH + 1], BF16, tag=f"v{t}", name=f"v{t}")
                for t in range(16)
            ]
            onorm = [
                onp.tile([128, T], BF16, tag=f"on{hp}", name=f"on{hp}")
                for hp in range(4)
            ]
            xts = [None, None]

            # ---- projection / oproj group emitters ----
            def x_load(th):
                set_label(f"xload{th}")
                xt = xtp.tile([128, 8, T // 2], BF16, tag="xt", name=f"xt{th}")
                t0 = th * (T // 2)
                for dt in range(8):
                    nc.sync.dma_start(xt[:, dt, :], xT_d[:, dt, t0 : t0 + T // 2])
                xts[th] = xt

            def j_prep(th, j):
                set_label(f"wload")
                w = wp.tile([128, 8, 128], BF16, tag="wqk")
                nc.sync.dma_start(w[:], wqk_d[:, j, :, :])
                return w

            def j_exec(th, j, tc_, w):
                set_label(f"qkv{th}")
                xt = xts[th]
                ps = psp.tile([128, 512], F32, tag="ps")
                for dt in range(8):
                    nc.tensor.matmul(
                        ps[:],
                        w[:, dt, :],
                        xt[:, dt, 512 * tc_ : 512 * (tc_ + 1)],
                        start=(dt == 0),
                        stop=(dt == 7),
                    )
                nc.vector.tensor_scalar_add(
                    qk_sb[j][
                        :, 1024 * th + 512 * tc_ : 1024 * th + 512 * (tc_ + 1)
                    ],
                    ps[:],
                    bqk_sb[:, j : j + 1],
                )

            def v_exec(th, tt):
                set_label(f"vproj{th}")
                xt = xts[th]
                tg = th * 8 + tt
                ps = psp.tile([128, 512], F32, tag="ps")
                for dt in range(8):
                    nc.tensor.matmul(
                        ps[:],
                        xt[:, dt, 128 * tt : 128 * (tt + 1)],
                        wv_sb[:, dt, :],
                        start=(dt == 0),
                        stop=(dt == 7),
                    )
                nc.vector.tensor_tensor(
                    vaug[tg][:, :, 0:DH],
                    ps[:].rearrange("p (h d) -> p h d", h=HPC),
                    bvb_sb[:].rearrange("p (h d) -> p h d", h=HPC),
                    mybir.AluOpType.add,
                )
                nc.gpsimd.memset(vaug[tg][:, :, DH : DH + 1], 1.0)

            def oproj_exec(c, g):
                set_label("oproj")
                qt = 4 * c + g // 2
                dc = g % 2
                ps = psp.tile([128, 512], F32, tag="ps")
                for hp in range(4):
                    nc.tensor.matmul(
                        ps[:],
                        onorm[hp][:, 128 * qt : 128 * (qt + 1)],
                        wo_sb[:, hp, 512 * dc : 512 * (dc + 1)],
                        start=(hp == 0),
                        stop=(hp == 3),
                    )
                osb = osbp.tile([128, 512], F32, tag="osb")
                nc.vector.tensor_copy(out=osb[:], in_=ps[:])
                nc.sync.dma_start(
                    out_d[128 * qt : 128 * (qt + 1), 512 * dc : 512 * (dc + 1)],
                    osb[:],
                )

            # filler plumbing: each filler is (prep_fn -> state, exec_fn)
            def J(th, j, tc_):
                return (lambda: j_prep(th, j), lambda w: j_exec(th, j, tc_, w))

            def V(th, tt):
                return (lambda: None, lambda _: v_exec(th, tt))

            def OP(c, g):
                return (lambda: None, lambda _: oproj_exec(c, g))

            # ---- attention chunk as a slot generator ----
            pending_norms = []

            def attn_slots(hp, c):
                qT = qk_sb[hp]
                kT = qk_sb[4 + hp]
                q0 = QC * c
                ktiles = 4 * (c + 1)
                set_label("attn")
                oA = psp.tile([128, 512], F32, tag="ps", name=f"oA{hp}_{c}")
                oB = psp.tile([128, 512], F32, tag="ps", name=f"oB{hp}_{c}")
                pts = {}

                def scores_exp(t):
                    set_label("attn")
                    j = t - 4 * c
                    qs = 128 * j if j >= 0 else 0
                    sp = spp.tile([128, 1024], F32, tag="sp")
                    for half, base in ((0, 0), (1, 64)):
                        nc.tensor.matmul(
                            sp[:, 512 * half + qs : 512 * (half + 1)],
                            kT[base : base + 64, 128 * t : 128 * (t + 1)],
                            qT[base : base + 64, q0 + qs : q0 + QC],
                            start=True,
                            stop=True,
                            tile_position=(base, 0),
                        )
                    pt = ptp.tile([128, 1024], BF16, tag="pt")
                    pts[t] = (pt, qs)
                    set_label("exp")
                    if qs == 0:
                        if use_dve_exp(hp, c, t):
                            nc.vector.tensor_scalar(
                                pt[:].bitcast(I16),
                                sp[:],
                                A_EXP,
                                B_EXP,
                                op0=mybir.AluOpType.mult,
                                op1=mybir.AluOpType.add,
                            )
                        else:
                            nc.scalar.activation(pt[:], sp[:], EXP)
                    else:
                        spv = sp[:].rearrange("p (h q) -> p h q", h=2)[:, :, qs:512]
                        ptv = pt[:].rearrange("p (h q) -> p h q", h=2)[:, :, qs:512]
                        nc.scalar.activation(ptv, spv, EXP)
                    if j >= 0:
                        ptv = pt[:].rearrange("p (h q) -> p h q", h=2)[
                            :, :, qs : qs + 128
                        ]
                        nc.gpsimd.tensor_tensor(
                            ptv, ptv, mask_sb[:], mybir.AluOpType.mult
                        )

                def av(t):
                    set_label("attn")
                    pt, qs = pts.pop(t)
                    for o_ps, half in ((oA, 0), (oB, 1)):
                        nc.tensor.matmul(
                            o_ps[0 : DH + 1, qs:512],
                            vaug[t][:, 2 * hp + half, :],
                            pt[:, 512 * half + qs : 512 * (half + 1)],
                            start=(t == 0),
                            stop=(t == ktiles - 1),
                            skip_group_check=True,
                        )

                scores_exp(0)
                yield
                for t in range(1, ktiles):
                    scores_exp(t)
                    av(t - 1)
                    yield
                av(ktiles - 1)

                # ---- normalize: drain psum early, reshape-reciprocal ----
                set_label("norm")
                ridx = hp * 4 + c
                ocps = []
                for o_ps in (oA, oB):
                    ocp = ocpp.tile([DH + 1, 512], F32, tag="ocp")
                    nc.vector.tensor_copy(out=ocp[:], in_=o_ps[0 : DH + 1, :])
                    ocps.append(ocp)
                for half, ocp in enumerate(ocps):
                    nc.sync.dma_start(
                        rsum_d[ridx : ridx + 1, 512 * half : 512 * (half + 1)],
                        ocp[DH : DH + 1, :],
                    )
                r8 = smallp.tile([128, 8], F32, tag="r8")
                nc.sync.dma_start(
                    r8[:], bass.AP(rsum_d, ridx * 1024, [[8, 128], [1, 8]])
                )
                rc8 = smallp.tile([128, 8], F32, tag="rc8")
                nc.vector.reciprocal(rc8[:], r8[:])
                nc.sync.dma_start(
                    bass.AP(rrec_d, ridx * 1024, [[8, 128], [1, 8]]), rc8[:]
                )

                # Phase 2 (broadcast + multiply) is DEFERRED into the next
                # chunk's slots so the in-order DVE stream never blocks on
                # the DMA round-trip latency.
                def finish_norm():
                    set_label("norm")
                    for half, ocp in enumerate(ocps):
                        bc = smallp.tile([64, 512], F32, tag="bc")
                        nc.sync.dma_start(
                            bc[:],
                            bass.AP(
                                rrec_d,
                                ridx * 1024 + 512 * half,
                                [[0, 64], [1, 512]],
                            ),
                        )
                        nc.vector.tensor_tensor(
                            onorm[hp][64 * half : 64 * half + 64, q0 : q0 + QC],
                            ocp[0:DH, :],
                            bc[:],
                            mybir.AluOpType.mult,
                        )

                pending_norms.append(finish_norm)
                yield

            # ---- region scheduler: interleave fillers between slots ----
            def region(chunks, fillers):
                """fillers: list of (prep, exec) or (prep, exec, deadline).
                A filler with deadline d is guaranteed to execute before the
                region's global slot index d is emitted; the rest are paced
                evenly.  Emission order IS execution order per engine, so
                deadlines encode read-after-write requirements."""
                fillers = [f if len(f) == 3 else (*f, None) for f in fillers]
                states = []

                def prep_to(i):
                    while len(states) < min(i, len(fillers)):
                        states.append(fillers[len(states)][0]())

                n_slots = sum(4 * (c + 1) + 1 for hp, c in chunks)
                prep_to(2)
                executed = 0
                slot_i = 0

                def run_due():
                    nonlocal executed
                    while executed < len(fillers):
                        dl = fillers[executed][2]
                        pace = (len(fillers) * (slot_i + 1)) // n_slots
                        if (dl is not None and dl <= slot_i) or executed < pace:
                            prep_to(executed + 3)
                            fillers[executed][1](states[executed])
                            executed += 1
                        else:
                            break

                for hp, c in chunks:
                    g = attn_slots(hp, c)
                    in_chunk = 0
                    while True:
                        run_due()
                        if in_chunk == 4:
                            while pending_norms:
                                pending_norms.pop(0)()
                        try:
                            next(g)
                        except StopIteration:
                            break
                        slot_i += 1
                        in_chunk += 1
                while executed < len(fillers):
                    prep_to(executed + 3)
                    fillers[executed][1](states[executed])
                    executed += 1
                while pending_norms:
                    pending_norms.pop(0)()

            # ---- schedule ----
            set_label("const")
            w00 = j_prep(0, 0)
            bqk_sb = constp.tile([128, 8], F32, tag="bqk")
            nc.sync.dma_start(bqk_sb[:], bqk_d[:])
            x_load(0)
            w04 = j_prep(0, 4)
            wv_sb = constp.tile([128, 8, DPC], BF16, tag="wv")
            nc.sync.dma_start(wv_sb[:], wv_d[:])
            bvb_sb = constp.tile([128, DPC], F32, tag="bvb")
            nc.sync.dma_start(bvb_sb[:], bass.AP(bv_d, 0, [[0, 128], [1, DPC]]))
            mask_sb = constp.tile([128, 2, 128], BF16, tag="mask")
            nc.sync.dma_start(
                mask_sb[:], mask_d[:].rearrange("p (h q) -> p h q", h=2)
            )

            # head: unlock (0, 0) only: Q/K tc0 j0,j4 + V tile 0
            j_exec(0, 0, 0, w00)
            j_exec(0, 4, 0, w04)
            v_exec(0, 0)

            wo_sb = constp.tile([128, 4, D], BF16, tag="wo")
            nc.sync.dma_start(wo_sb[:], wo_d[:])

            # chunk slot spans: c0 6/chunk, c1 10, c2 14, c3 18
            region(
                [(hp, 0) for hp in range(4)],
                [
                    (*V(0, 1), 1), (*V(0, 2), 2), (*V(0, 3), 3),
                    (*J(0, 1, 0), 4), (*J(0, 5, 0), 4), J(0, 0, 1), J(0, 4, 1),
                    (*J(0, 2, 0), 9), (*J(0, 6, 0), 9), J(0, 1, 1), J(0, 5, 1),
                    (*J(0, 3, 0), 14), (*J(0, 7, 0), 14), V(0, 4), V(0, 5),
                ],
            )
            x_load(1)
            region(
                [(hp, 1) for hp in range(4)],
                [
                    (*V(0, 6), 6), (*V(0, 7), 7), J(1, 0, 0), J(1, 4, 0),
                    (*J(0, 2, 1), 16), (*J(0, 6, 1), 16), OP(0, 0), OP(0, 1),
                    J(1, 1, 0), J(1, 5, 0),
                    (*J(0, 3, 1), 25), (*J(0, 7, 1), 25), OP(0, 2), OP(0, 3),
                    J(1, 2, 0), J(1, 6, 0), J(1, 3, 0), J(1, 7, 0),
                    OP(0, 4), OP(0, 5),
                ],
            )
            region(
                [(hp, 2) for hp in range(4)],
                [
                    (*V(1, 0), 7), (*V(1, 1), 8), (*V(1, 2), 9), (*V(1, 3), 10),
                    OP(0, 6), OP(0, 7), J(1, 0, 1), J(1, 4, 1),
                    OP(1, 0), OP(1, 1), J(1, 1, 1), J(1, 5, 1),
                    OP(1, 2), OP(1, 3), J(1, 2, 1), J(1, 6, 1),
                ],
            )
            region(
                [(hp, 3) for hp in range(4)],
                [
                    (*V(1, 4), 11), (*V(1, 5), 12), (*V(1, 6), 13),
                    (*V(1, 7), 14), (*J(1, 3, 1), 48), (*J(1, 7, 1), 48),
                    OP(1, 4), OP(1, 5), OP(1, 6), OP(1, 7),
                    OP(2, 0), OP(2, 1), OP(2, 2), OP(2, 3),
                ],
            )
            # tail: oproj chunk-2 remainder overlaps the final norm's DRAM
            # round trip, then chunk-3 oproj with the head-pair-3 matmuls
            # batched last so hp0-2 accumulation also runs inside that window.
            set_label("oproj")
            for g in range(4, 8):
                oproj_exec(2, g)
            for gg in (0, 4):
                groups = []
                for g in range(gg, gg + 4):
                    qt = 12 + g // 2
                    dc = g % 2
                    ps = psp.tile([128, 512], F32, tag="ps")
                    groups.append((qt, dc, ps))
                    for hp in range(3):
                        nc.tensor.matmul(
                            ps[:],
                            onorm[hp][:, 128 * qt : 128 * (qt + 1)],
                            wo_sb[:, hp, 512 * dc : 512 * (dc + 1)],
                            start=(hp == 0),
                            stop=False,
                            skip_group_check=True,
                        )
                for qt, dc, ps in groups:
                    nc.tensor.matmul(
                        ps[:],
                        onorm[3][:, 128 * qt : 128 * (qt + 1)],
                        wo_sb[:, 3, 512 * dc : 512 * (dc + 1)],
                        start=False,
                        stop=True,
                        skip_group_check=True,
                    )
                    osb = osbp.tile([128, 512], F32, tag="osb")
                    nc.vector.tensor_copy(out=osb[:], in_=ps[:])
                    nc.sync.dma_start(
                        out_d[128 * qt : 128 * (qt + 1), 512 * dc : 512 * (dc + 1)],
                        osb[:],
                    )

    split_excess_waits(nc)
    return nc


TRACE = False
LAST_EXEC_NS = None

_NC = None


def _get_nc():
    global _NC
    if _NC is None:
        _NC = build()
    return _NC


def kernel(x, W_qkv, b_qkv, W_o, b_o):
    x = np.asarray(x, dtype=np.float32)
    W_qkv = np.asarray(W_qkv, dtype=np.float32)
    b_qkv = np.asarray(b_qkv, dtype=np.float32)
    W_o = np.asarray(W_o, dtype=np.float32)
    b_o = np.asarray(b_o, dtype=np.float32)
    import ml_dtypes

    bf = ml_dtypes.bfloat16
    scale = 1.0 / np.sqrt(np.float32(DH))

    # x^T per batch, partition-major [p, dt, t] (shared between 2 cores)
    xTs = [
        np.ascontiguousarray(
            x[b].T.reshape(8, 128, T).transpose(1, 0, 2).astype(bf)
        )
        for b in range(B)
    ]

    # causal mask tile: keep iff q-local >= k-local
    tri1 = np.triu(np.ones((128, 128), np.float32))
    tri = np.concatenate([tri1, tri1], axis=1).astype(bf)

    in_maps = []
    for c in range(N_CORES):
        b, g = divmod(c, 2)
        h0 = g * HPC
        qcols = slice(h0 * DH, h0 * DH + DPC)
        kcols = slice(D + h0 * DH, D + h0 * DH + DPC)
        vcols = slice(2 * D + h0 * DH, 2 * D + h0 * DH + DPC)
        wqk = np.concatenate(
            [W_qkv[:, qcols] * scale, W_qkv[:, kcols]], axis=1
        )  # [D, 1024]
        wqk_p = wqk.reshape(8, 128, 8, 128).transpose(1, 2, 0, 3).astype(bf)
        wv_p = (
            W_qkv[:, vcols].reshape(8, 128, DPC).transpose(1, 0, 2).astype(bf)
        )
        wo_p = (
            W_o[g * DPC : (g + 1) * DPC, :]
            .reshape(4, 128, D)
            .transpose(1, 0, 2)
            .astype(bf)
        )
        bqk = np.concatenate([b_qkv[qcols] * scale, b_qkv[kcols]]).astype(
            np.float32
        )
        in_maps.append(
            {
                "xT": xTs[b],
                "wqk": np.ascontiguousarray(wqk_p),
                "wv": np.ascontiguousarray(wv_p),
                "wo": np.ascontiguousarray(wo_p),
                "bqk": np.ascontiguousarray(bqk.reshape(8, 128).T),
                "bv": np.ascontiguousarray(
                    b_qkv[vcols].reshape(1, DPC).astype(np.float32)
                ),
                "trimask": tri,
            }
        )

    nc = _get_nc()
    global LAST_EXEC_NS
    res = None
    last_err = None
    for attempt in range(3):
        try:
            res = run_bass_kernel_spmd(
                nc, in_maps, list(range(N_CORES)), trace=TRACE
            )
            break
        except Exception as e:  # transient device wedge: retry
            last_err = e
            import time as _time

            _time.sleep(5)
    if res is None:
        raise last_err
    LAST_EXEC_NS = res.exec_time_ns
    globals()["_LAST_RES"] = res
    parts = [res.results[c]["out"] for c in range(N_CORES)]
    out = np.empty((B, T, D), np.float32)
    for b in range(B):
        out[b] = parts[2 * b] + parts[2 * b + 1] + b_o[None, :]
    return out


# revision 19
# speedup vs baseline: 1.0449x; 1.0449x over previous
"""Causal multi-head attention (B=4, T=2048, D=1024, H=16) on 8 TRN2 NeuronCores.

Sharding: core c -> batch b = c // 2, head-group g = c % 2 (8 heads each).
Host pre-packs x^T per batch and W_qkv/W_o/biases per head-group into
partition-major bf16 layouts (1/sqrt(dh) folded into W_q/b_q on host).

Engines execute their instruction streams in order, so the build explicitly
software-pipelines: attention chunks (hp, c) emit slot-by-slot (scores ->
exp -> mask -> AV with 1-slot skew) and projection / output-projection
"filler" groups are interleaved between slots, paced evenly per region:

  head:  Q/K j0,j4 (tc0) and V tiles 0-3 for q/k range [0,512)
  c=0:   remaining th0 j-groups
  c=1:   V tiles 4-7, th1 j-groups (tc0), output projection of chunk 0
  c=2:   V tiles 8-11, th1 j-groups (tc1), oproj chunk 1
  c=3:   V tiles 12-15, oproj chunk 2;  tail: oproj chunk 3

Attention per slot: S^T = K^T.T Q^T (two heads row-packed, one 2-bank psum
tile), exp on ScalarE -> P^T bf16 (a tunable subset of off-diagonal slots
instead computes exp on VectorE via the Schraudolph bit trick: bf16 bits =
round(s*128*log2e + 16256 - 5.5), one tensor_scalar into an int16 view),
causal via suffix-trimming + triangular mask multiply on GpSimd, then
o^T (+rowsum via the ones-augmented V) accumulates in psum.  At chunk end
o^T is copied out of psum immediately (frees the accumulator), the rowsum
pair round-trips through DRAM as [128,8] so the exact DVE reciprocal runs
wide, and a second round trip broadcasts it for the normalize multiply.

Host sums the two head-group partials per batch and adds b_o.
"""

import sys

sys.path.insert(0, "/opt/trn_rl_repo")

import numpy as np

import concourse.bass as bass
import concourse.mybir as mybir
from concourse.bass_utils import run_bass_kernel_spmd
from concourse.tile import TileContext

F32 = mybir.dt.float32
BF16 = mybir.dt.bfloat16
I16 = mybir.dt.int16
EXP = mybir.ActivationFunctionType.Exp
MULT = None  # set after mybir import below

B, T, D, H = 4, 2048, 1024, 16
DH = D // H          # 64
HPC = H // 2         # heads per core = 8
DPC = HPC * DH       # 512 projected dims per core
N_CORES = 8
QC = 512             # q-chunk width in phase 2
KT = 128             # k-tile width

# Schraudolph fast-exp constants (bf16 bits = round(s*A + B))
A_EXP = 128.0 * 1.4426950408889634
B_EXP = 127.0 * 128.0 - 5.5


def use_dve_exp(hp, c, t):
    """Offload this slot's exp to VectorE? Only off-diagonal (unmasked,
    untrimmed) slots in the late ACT-bound regions."""
    if t >= 4 * c:          # diagonal-block tile
        return False
    return c >= 2 and t % 2 == 1


def split_excess_waits(nc, cap=1):
    """walrus limits sync-wait slots per ISA instruction (1 for several
    structs).  Move excess waits onto InstEventSemaphore instructions
    inserted just before the offender on the same engine."""
    n_split = 0
    for f in nc.m.functions:
        for blk in f.blocks:
            insts = blk.instructions
            out = []
            changed = False
            for inst in insts:
                si = inst.sync_info
                waits = list(si.on_wait) if si is not None else []
                if len(waits) > cap:
                    for j, w in enumerate(waits[:-cap]):
                        ev = mybir.InstEventSemaphore(
                            name=f"{inst.name}-w{j}", ins=[], outs=[]
                        )
                        ev.engine = inst.engine
                        ev.sync_info = mybir.SyncInfo(on_wait=[w], on_update=[])
                        out.append(ev)
                        n_split += 1
                    inst.sync_info = mybir.SyncInfo(
                        on_wait=waits[-cap:], on_update=list(si.on_update)
                    )
                    changed = True
                out.append(inst)
            if changed:
                blk.instructions = out
    return n_split


INST_LABELS = {}


def build():
    nc = bass.Bass(target_bir_lowering=False)

    _label = ["init"]

    def set_label(s):
        _label[0] = s

    for eng in (nc.tensor, nc.vector, nc.scalar, nc.gpsimd, nc.sync):
        orig = eng.add_instruction

        def wrapped(inst, _orig=orig):
            r = _orig(inst)
            try:
                INST_LABELS[inst.name] = _label[0]
            except Exception:
                pass
            return r

        eng.add_instruction = wrapped

    xT_d = nc.dram_tensor("xT", [128, 8, T], BF16, kind="ExternalInput")
    wqk_d = nc.dram_tensor("wqk", [128, 8, 8, 128], BF16, kind="ExternalInput")
    wv_d = nc.dram_tensor("wv", [128, 8, DPC], BF16, kind="ExternalInput")
    wo_d = nc.dram_tensor("wo", [128, 4, D], BF16, kind="ExternalInput")
    bqk_d = nc.dram_tensor("bqk", [128, 8], F32, kind="ExternalInput")
    bv_d = nc.dram_tensor("bv", [1, DPC], F32, kind="ExternalInput")
    mask_d = nc.dram_tensor("trimask", [128, 256], BF16, kind="ExternalInput")
    out_d = nc.dram_tensor("out", [T, D], F32, kind="ExternalOutput")
    rsum_d = nc.dram_tensor("rsum", [16, 1024], F32)  # internal scratch
    rrec_d = nc.dram_tensor("rrec", [16, 1024], F32)  # internal scratch

    with TileContext(nc) as tc:
        with (
            tc.tile_pool(name="const", bufs=1) as constp,
            tc.tile_pool(name="wstream", bufs=4) as wp,
            tc.tile_pool(name="xt", bufs=2) as xtp,
            tc.tile_pool(name="qk", bufs=1) as qkp,
            tc.tile_pool(name="vaug", bufs=1) as vp,
            tc.tile_pool(name="onorm", bufs=1) as onp,
            tc.tile_pool(name="pt", bufs=6) as ptp,
            tc.tile_pool(name="ocp", bufs=6) as ocpp,
            tc.tile_pool(name="small", bufs=8) as smallp,
            tc.tile_pool(name="osb", bufs=3) as osbp,
            tc.tile_pool(name="ps", bufs=4, space="PSUM") as psp,
            tc.tile_pool(name="spair", bufs=2, space="PSUM") as spp,
        ):
            # ---- persistent tiles ----
            qk_sb = [
                qkp.tile([128, T], BF16, tag=f"qk{j}", name=f"qk{j}")
                for j in range(8)
            ]
            vaug = [
                vp.tile([128, HPC, DH + 1], BF16, tag=f"v{t}", name=f"v{t}")
                for t in range(16)
            ]
            onorm = [
                onp.tile([128, T], BF16, tag=f"on{hp}", name=f"on{hp}")
                for hp in range(4)
            ]
            xts = [None, None]

            # ---- projection / oproj group emitters ----
            def x_load(th):
                set_label(f"xload{th}")
                xt = xtp.tile([128, 8, T // 2], BF16, tag="xt", name=f"xt{th}")
                t0 = th * (T // 2)
                for dt in range(8):
                    nc.sync.dma_start(xt[:, dt, :], xT_d[:, dt, t0 : t0 + T // 2])
                xts[th] = xt

            def j_prep(th, j):
                set_label(f"wload")
                w = wp.tile([128, 8, 128], BF16, tag="wqk")
                nc.sync.dma_start(w[:], wqk_d[:, j, :, :])
                return w

            def j_exec(th, j, tc_, w):
                set_label(f"qkv{th}")
                xt = xts[th]
                ps = psp.tile([128, 512], F32, tag="ps")
                for dt in range(8):
                    nc.tensor.matmul(
                        ps[:],
                        w[:, dt, :],
                        xt[:, dt, 512 * tc_ : 512 * (tc_ + 1)],
                        start=(dt == 0),
                        stop=(dt == 7),
                    )
                nc.vector.tensor_scalar_add(
                    qk_sb[j][
                        :, 1024 * th + 512 * tc_ : 1024 * th + 512 * (tc_ + 1)
                    ],
                    ps[:],
                    bqk_sb[:, j : j + 1],
                )

            def v_exec(th, tt):
                set_label(f"vproj{th}")
                xt = xts[th]
                tg = th * 8 + tt
                ps = psp.tile([128, 512], F32, tag="ps")
                for dt in range(8):
                    nc.tensor.matmul(
                        ps[:],
                        xt[:, dt, 128 * tt : 128 * (tt + 1)],
                        wv_sb[:, dt, :],
                        start=(dt == 0),
                        stop=(dt == 7),
                    )
                nc.vector.tensor_tensor(
                    vaug[tg][:, :, 0:DH],
                    ps[:].rearrange("p (h d) -> p h d", h=HPC),
                    bvb_sb[:].rearrange("p (h d) -> p h d", h=HPC),
                    mybir.AluOpType.add,
                )
                nc.gpsimd.memset(vaug[tg][:, :, DH : DH + 1], 1.0)

            def oproj_exec(c, g):
                set_label("oproj")
                qt = 4 * c + g // 2
                dc = g % 2
                ps = psp.tile([128, 512], F32, tag="ps")
                for hp in range(4):
                    nc.tensor.matmul(
                        ps[:],
                        onorm[hp][:, 128 * qt : 128 * (qt + 1)],
                        wo_sb[:, hp, 512 * dc : 512 * (dc + 1)],
                        start=(hp == 0),
                        stop=(hp == 3),
                    )
                osb = osbp.tile([128, 512], F32, tag="osb")
                nc.vector.tensor_copy(out=osb[:], in_=ps[:])
                nc.sync.dma_start(
                    out_d[128 * qt : 128 * (qt + 1), 512 * dc : 512 * (dc + 1)],
                    osb[:],
                )

            # filler plumbing: each filler is (prep_fn -> state, exec_fn)
            def J(th, j, tc_):
                return (lambda: j_prep(th, j), lambda w: j_exec(th, j, tc_, w))

            def V(th, tt):
                return (lambda: None, lambda _: v_exec(th, tt))

            def OP(c, g):
                return (lambda: None, lambda _: oproj_exec(c, g))

            # ---- attention chunk as a slot generator ----
            pending_norms = []

            def attn_slots(hp, c):
                qT = qk_sb[hp]
                kT = qk_sb[4 + hp]
                q0 = QC * c
                ktiles = 4 * (c + 1)
                set_label("attn")
                oA = psp.tile([128, 512], F32, tag="ps", name=f"oA{hp}_{c}")
                oB = psp.tile([128, 512], F32, tag="ps", name=f"oB{hp}_{c}")
                pts = {}

                def scores_exp(t):
                    set_label("attn")
                    j = t - 4 * c
                    qs = 128 * j if j >= 0 else 0
                    sp = spp.tile([128, 1024], F32, tag="sp")
                    for half, base in ((0, 0), (1, 64)):
                        nc.tensor.matmul(
                            sp[:, 512 * half + qs : 512 * (half + 1)],
                            kT[base : base + 64, 128 * t : 128 * (t + 1)],
                            qT[base : base + 64, q0 + qs : q0 + QC],
                            start=True,
                            stop=True,
                            tile_position=(base, 0),
                        )
                    pt = ptp.tile([128, 1024], BF16, tag="pt")
                    pts[t] = (pt, qs)
                    set_label("exp")
                    if qs == 0:
                        if use_dve_exp(hp, c, t):
                            nc.vector.tensor_scalar(
                                pt[:].bitcast(I16),
                                sp[:],
                                A_EXP,
                                B_EXP,
                                op0=mybir.AluOpType.mult,
                                op1=mybir.AluOpType.add,
                            )
                        else:
                            nc.scalar.activation(pt[:], sp[:], EXP)
                    else:
                        spv = sp[:].rearrange("p (h q) -> p h q", h=2)[:, :, qs:512]
                        ptv = pt[:].rearrange("p (h q) -> p h q", h=2)[:, :, qs:512]
                        nc.scalar.activation(ptv, spv, EXP)
                    if j >= 0:
                        ptv = pt[:].rearrange("p (h q) -> p h q", h=2)[
                            :, :, qs : qs + 128
                        ]
                        nc.gpsimd.tensor_tensor(
                            ptv, ptv, mask_sb[:], mybir.AluOpType.mult
                        )

                def av(t):
                    set_label("attn")
                    pt, qs = pts.pop(t)
                    for o_ps, half in ((oA, 0), (oB, 1)):
                        nc.tensor.matmul(
                            o_ps[0 : DH + 1, qs:512],
                            vaug[t][:, 2 * hp + half, :],
                            pt[:, 512 * half + qs : 512 * (half + 1)],
                            start=(t == 0),
                            stop=(t == ktiles - 1),
                            skip_group_check=True,
                        )

                scores_exp(0)
                yield
                for t in range(1, ktiles):
                    scores_exp(t)
                    av(t - 1)
                    yield
                av(ktiles - 1)

                # ---- normalize: drain psum early, reshape-reciprocal ----
                set_label("norm")
                ridx = hp * 4 + c
                ocps = []
                for o_ps in (oA, oB):
                    ocp = ocpp.tile([DH + 1, 512], F32, tag="ocp")
                    nc.vector.tensor_copy(out=ocp[:], in_=o_ps[0 : DH + 1, :])
                    ocps.append(ocp)
                for half, ocp in enumerate(ocps):
                    nc.sync.dma_start(
                        rsum_d[ridx : ridx + 1, 512 * half : 512 * (half + 1)],
                        ocp[DH : DH + 1, :],
                    )
                r8 = smallp.tile([128, 8], F32, tag="r8")
                nc.sync.dma_start(
                    r8[:], bass.AP(rsum_d, ridx * 1024, [[8, 128], [1, 8]])
                )
                rc8 = smallp.tile([128, 8], F32, tag="rc8")
                nc.vector.reciprocal(rc8[:], r8[:])
                nc.sync.dma_start(
                    bass.AP(rrec_d, ridx * 1024, [[8, 128], [1, 8]]), rc8[:]
                )

                # Phase 2 (broadcast + multiply) is DEFERRED into the next
                # chunk's slots so the in-order DVE stream never blocks on
                # the DMA round-trip latency.
                def finish_norm():
                    set_label("norm")
                    for half, ocp in enumerate(ocps):
                        bc = smallp.tile([64, 512], F32, tag="bc")
                        nc.sync.dma_start(
                            bc[:],
                            bass.AP(
                                rrec_d,
                                ridx * 1024 + 512 * half,
                                [[0, 64], [1, 512]],
                            ),
                        )
                        nc.vector.tensor_tensor(
                            onorm[hp][64 * half : 64 * half + 64, q0 : q0 + QC],
                            ocp[0:DH, :],
                            bc[:],
                            mybir.AluOpType.mult,
                        )

                pending_norms.append(finish_norm)
                yield

            # ---- region scheduler: interleave fillers between slots ----
            def region(chunks, fillers):
                """fillers: list of (prep, exec) or (prep, exec, deadline).
                A filler with deadline d is guaranteed to execute before the
                region's global slot index d is emitted; the rest are paced
                evenly.  Emission order IS execution order per engine, so
                deadlines encode read-after-write requirements."""
                fillers = [f if len(f) == 3 else (*f, None) for f in fillers]
                states = []

                def prep_to(i):
                    while len(states) < min(i, len(fillers)):
                        states.append(fillers[len(states)][0]())

                n_slots = sum(4 * (c + 1) + 1 for hp, c in chunks)
                prep_to(2)
                executed = 0
                slot_i = 0

                def run_due():
                    nonlocal executed
                    while executed < len(fillers):
                        dl = fillers[executed][2]
                        pace = (len(fillers) * (slot_i + 1)) // n_slots
                        if (dl is not None and dl <= slot_i) or executed < pace:
                            prep_to(executed + 3)
                            fillers[executed][1](states[executed])
                            executed += 1
                        else:
                            break

                for hp, c in chunks:
                    g = attn_slots(hp, c)
                    in_chunk = 0
                    while True:
                        run_due()
                        if in_chunk == 4:
                            while pending_norms:
                                pending_norms.pop(0)()
                        try:
                            next(g)
                        except StopIteration:
                            break
                        slot_i += 1
                        in_chunk += 1
                while executed < len(fillers):
                    prep_to(executed + 3)
                    fillers[executed][1](states[executed])
                    executed += 1
                while pending_norms:
                    pending_norms.pop(0)()

            # ---- schedule ----
            set_label("const")
            w00 = j_prep(0, 0)
            bqk_sb = constp.tile([128, 8], F32, tag="bqk")
            nc.sync.dma_start(bqk_sb[:], bqk_d[:])
            x_load(0)
            w04 = j_prep(0, 4)
            wv_sb = constp.tile([128, 8, DPC], BF16, tag="wv")
            nc.sync.dma_start(wv_sb[:], wv_d[:])
            bvb_sb = constp.tile([128, DPC], F32, tag="bvb")
            nc.sync.dma_start(bvb_sb[:], bass.AP(bv_d, 0, [[0, 128], [1, DPC]]))
            mask_sb = constp.tile([128, 2, 128], BF16, tag="mask")
            nc.sync.dma_start(
                mask_sb[:], mask_d[:].rearrange("p (h q) -> p h q", h=2)
            )

            # head: unlock (0, 0) only: Q/K tc0 j0,j4 + V tile 0
            j_exec(0, 0, 0, w00)
            j_exec(0, 4, 0, w04)
            v_exec(0, 0)

            wo_sb = constp.tile([128, 4, D], BF16, tag="wo")
            nc.sync.dma_start(wo_sb[:], wo_d[:])

            # chunk slot spans: c0 6/chunk, c1 10, c2 14, c3 18
            region(
                [(hp, 0) for hp in range(4)],
                [
                    (*V(0, 1), 1), (*V(0, 2), 2), (*V(0, 3), 3),
                    (*J(0, 1, 0), 4), (*J(0, 5, 0), 4), J(0, 0, 1), J(0, 4, 1),
                    (*J(0, 2, 0), 9), (*J(0, 6, 0), 9), J(0, 1, 1), J(0, 5, 1),
                    (*J(0, 3, 0), 14), (*J(0, 7, 0), 14), V(0, 4), V(0, 5),
                ],
            )
            x_load(1)
            region(
                [(hp, 1) for hp in range(4)],
                [
                    (*V(0, 6), 6), (*V(0, 7), 7), J(1, 0, 0), J(1, 4, 0),
                    (*J(0, 2, 1), 16), (*J(0, 6, 1), 16), OP(0, 0), OP(0, 1),
                    J(1, 1, 0), J(1, 5, 0),
                    (*J(0, 3, 1), 25), (*J(0, 7, 1), 25), OP(0, 2), OP(0, 3),
                    J(1, 2, 0), J(1, 6, 0), J(1, 3, 0), J(1, 7, 0),
                    OP(0, 4), OP(0, 5),
                ],
            )
            region(
                [(hp, 2) for hp in range(4)],
                [
                    (*V(1, 0), 7), (*V(1, 1), 8), (*V(1, 2), 9), (*V(1, 3), 10),
                    OP(0, 6), OP(0, 7), J(1, 0, 1), J(1, 4, 1),
                    OP(1, 0), OP(1, 1), J(1, 1, 1), J(1, 5, 1),
                    OP(1, 2), OP(1, 3), J(1, 2, 1), J(1, 6, 1),
                ],
            )
            region(
                [(hp, 3) for hp in range(4)],
                [
                    (*V(1, 4), 11), (*V(1, 5), 12), (*V(1, 6), 13),
                    (*V(1, 7), 14), (*J(1, 3, 1), 48), (*J(1, 7, 1), 48),
                    OP(1, 4), OP(1, 5), OP(1, 6), OP(1, 7),
                    OP(2, 0), OP(2, 1), OP(2, 2), OP(2, 3),
                ],
            )
            # tail: oproj chunk-2 remainder overlaps the final norm's DRAM
            # round trip, then chunk-3 oproj with the head-pair-3 matmuls
            # batched last so hp0-2 accumulation also runs inside that window.
            set_label("oproj")
            for g in range(4, 8):
                oproj_exec(2, g)
            for gg in (0, 4):
                groups = []
                for g in range(gg, gg + 4):
                    qt = 12 + g // 2
                    dc = g % 2
                    ps = psp.tile([128, 512], F32, tag="ps")
                    groups.append((qt, dc, ps))
                    for hp in range(3):
                        nc.tensor.matmul(
                            ps[:],
                            onorm[hp][:, 128 * qt : 128 * (qt + 1)],
                            wo_sb[:, hp, 512 * dc : 512 * (dc + 1)],
                            start=(hp == 0),
                            stop=False,
                            skip_group_check=True,
                        )
                for qt, dc, ps in groups:
                    nc.tensor.matmul(
                        ps[:],
                        onorm[3][:, 128 * qt : 128 * (qt + 1)],
                        wo_sb[:, 3, 512 * dc : 512 * (dc + 1)],
                        start=False,
                        stop=True,
                        skip_group_check=True,
                    )
                    osb = osbp.tile([128, 512], F32, tag="osb")
                    nc.vector.tensor_copy(out=osb[:], in_=ps[:])
                    nc.sync.dma_start(
                        out_d[128 * qt : 128 * (qt + 1), 512 * dc : 512 * (dc + 1)],
                        osb[:],
                    )

    split_excess_waits(nc)
    return nc


TRACE = False
LAST_EXEC_NS = None

_NC = None


def _get_nc():
    global _NC
    if _NC is None:
        _NC = build()
    return _NC


def kernel(x, W_qkv, b_qkv, W_o, b_o):
    x = np.asarray(x, dtype=np.float32)
    W_qkv = np.asarray(W_qkv, dtype=np.float32)
    b_qkv = np.asarray(b_qkv, dtype=np.float32)
    W_o = np.asarray(W_o, dtype=np.float32)
    b_o = np.asarray(b_o, dtype=np.float32)
    import ml_dtypes

    bf = ml_dtypes.bfloat16
    scale = 1.0 / np.sqrt(np.float32(DH))

    # x^T per batch, partition-major [p, dt, t] (shared between 2 cores)
    xTs = [
        np.ascontiguousarray(
            x[b].T.reshape(8, 128, T).transpose(1, 0, 2).astype(bf)
        )
        for b in range(B)
    ]

    # causal mask tile: keep iff q-local >= k-local
    tri1 = np.triu(np.ones((128, 128), np.float32))
    tri = np.concatenate([tri1, tri1], axis=1).astype(bf)

    in_maps = []
    for c in range(N_CORES):
        b, g = divmod(c, 2)
        h0 = g * HPC
        qcols = slice(h0 * DH, h0 * DH + DPC)
        kcols = slice(D + h0 * DH, D + h0 * DH + DPC)
        vcols = slice(2 * D + h0 * DH, 2 * D + h0 * DH + DPC)
        wqk = np.concatenate(
            [W_qkv[:, qcols] * scale, W_qkv[:, kcols]], axis=1
        )  # [D, 1024]
        wqk_p = wqk.reshape(8, 128, 8, 128).transpose(1, 2, 0, 3).astype(bf)
        wv_p = (
            W_qkv[:, vcols].reshape(8, 128, DPC).transpose(1, 0, 2).astype(bf)
        )
        wo_p = (
            W_o[g * DPC : (g + 1) * DPC, :]
            .reshape(4, 128, D)
            .transpose(1, 0, 2)
            .astype(bf)
        )
        bqk = np.concatenate([b_qkv[qcols] * scale, b_qkv[kcols]]).astype(
            np.float32
        )
        in_maps.append(
            {
                "xT": xTs[b],
                "wqk": np.ascontiguousarray(wqk_p),
                "wv": np.ascontiguousarray(wv_p),
                "wo": np.ascontiguousarray(wo_p),
                "bqk": np.ascontiguousarray(bqk.reshape(8, 128).T),
                "bv": np.ascontiguousarray(
                    b_qkv[vcols].reshape(1, DPC).astype(np.float32)
                ),
                "trimask": tri,
            }
        )

    nc = _get_nc()
    global LAST_EXEC_NS
    res = None
    last_err = None
    for attempt in range(3):
        try:
            res = run_bass_kernel_spmd(
                nc, in_maps, list(range(N_CORES)), trace=TRACE
            )
            break
        except Exception as e:  # transient device wedge: retry
            last_err = e
            import time as _time

            _time.sleep(5)
    if res is None:
        raise last_err
    LAST_EXEC_NS = res.exec_time_ns
    globals()["_LAST_RES"] = res
    parts = [res.results[c]["out"] for c in range(N_CORES)]
    out = np.empty((B, T, D), np.float32)
    for b in range(B):
        out[b] = parts[2 * b] + parts[2 * b + 1] + b_o[None, :]
    return out


# revision 20
# speedup vs baseline: 1.0550x; 1.0096x over previous
"""Causal multi-head attention (B=4, T=2048, D=1024, H=16) on 8 TRN2 NeuronCores.

Sharding: core c -> batch b = c // 2, head-group g = c % 2 (8 heads each).
Host pre-packs x^T per batch and W_qkv/W_o/biases per head-group into
partition-major bf16 layouts (1/sqrt(dh) folded into W_q/b_q on host).

Engines execute their instruction streams in order, so the build explicitly
software-pipelines: attention chunks (hp, c) emit slot-by-slot (scores ->
exp -> mask -> AV with 1-slot skew) and projection / output-projection
"filler" groups are interleaved between slots, paced evenly per region:

  head:  Q/K j0,j4 (tc0) and V tiles 0-3 for q/k range [0,512)
  c=0:   remaining th0 j-groups
  c=1:   V tiles 4-7, th1 j-groups (tc0), output projection of chunk 0
  c=2:   V tiles 8-11, th1 j-groups (tc1), oproj chunk 1
  c=3:   V tiles 12-15, oproj chunk 2;  tail: oproj chunk 3

Attention per slot: S^T = K^T.T Q^T (two heads row-packed, one 2-bank psum
tile), exp on ScalarE -> P^T bf16 (a tunable subset of off-diagonal slots
instead computes exp on VectorE via the Schraudolph bit trick: bf16 bits =
round(s*128*log2e + 16256 - 5.5), one tensor_scalar into an int16 view),
causal via suffix-trimming + triangular mask multiply on GpSimd, then
o^T (+rowsum via the ones-augmented V) accumulates in psum.  At chunk end
o^T is copied out of psum immediately (frees the accumulator), the rowsum
pair round-trips through DRAM as [128,8] so the exact DVE reciprocal runs
wide, and a second round trip broadcasts it for the normalize multiply.

Host sums the two head-group partials per batch and adds b_o.
"""

import sys

sys.path.insert(0, "/opt/trn_rl_repo")

import numpy as np

import concourse.bass as bass
import concourse.mybir as mybir
from concourse.bass_utils import run_bass_kernel_spmd
from concourse.tile import TileContext

F32 = mybir.dt.float32
BF16 = mybir.dt.bfloat16
I16 = mybir.dt.int16
EXP = mybir.ActivationFunctionType.Exp
MULT = None  # set after mybir import below

B, T, D, H = 4, 2048, 1024, 16
DH = D // H          # 64
HPC = H // 2         # heads per core = 8
DPC = HPC * DH       # 512 projected dims per core
N_CORES = 8
QC = 512             # q-chunk width in phase 2
KT = 128             # k-tile width

# Schraudolph fast-exp constants (bf16 bits = round(s*A + B))
A_EXP = 128.0 * 1.4426950408889634
B_EXP = 127.0 * 128.0 - 5.5


def use_dve_exp(hp, c, t):
    """Offload this slot's exp to VectorE? Only off-diagonal (unmasked,
    untrimmed) slots in the late ACT-bound regions."""
    if t >= 4 * c:          # diagonal-block tile
        return False
    if hp == 3 and c == 3:
        # final chunk: keep DVE free so the psum-draining copies run
        # promptly and the tail output projection is not starved
        return False
    return c >= 2 and t % 2 == 1


def split_excess_waits(nc, cap=1):
    """walrus limits sync-wait slots per ISA instruction (1 for several
    structs).  Move excess waits onto InstEventSemaphore instructions
    inserted just before the offender on the same engine."""
    n_split = 0
    for f in nc.m.functions:
        for blk in f.blocks:
            insts = blk.instructions
            out = []
            changed = False
            for inst in insts:
                si = inst.sync_info
                waits = list(si.on_wait) if si is not None else []
                if len(waits) > cap:
                    for j, w in enumerate(waits[:-cap]):
                        ev = mybir.InstEventSemaphore(
                            name=f"{inst.name}-w{j}", ins=[], outs=[]
                        )
                        ev.engine = inst.engine
                        ev.sync_info = mybir.SyncInfo(on_wait=[w], on_update=[])
                        out.append(ev)
                        n_split += 1
                    inst.sync_info = mybir.SyncInfo(
                        on_wait=waits[-cap:], on_update=list(si.on_update)
                    )
                    changed = True
                out.append(inst)
            if changed:
                blk.instructions = out
    return n_split


INST_LABELS = {}


def build():
    nc = bass.Bass(target_bir_lowering=False)

    _label = ["init"]

    def set_label(s):
        _label[0] = s

    for eng in (nc.tensor, nc.vector, nc.scalar, nc.gpsimd, nc.sync):
        orig = eng.add_instruction

        def wrapped(inst, _orig=orig):
            r = _orig(inst)
            try:
                INST_LABELS[inst.name] = _label[0]
            except Exception:
                pass
            return r

        eng.add_instruction = wrapped

    xT_d = nc.dram_tensor("xT", [128, 8, T], BF16, kind="ExternalInput")
    wqk_d = nc.dram_tensor("wqk", [128, 8, 8, 128], BF16, kind="ExternalInput")
    wv_d = nc.dram_tensor("wv", [128, 8, DPC], BF16, kind="ExternalInput")
    wo_d = nc.dram_tensor("wo", [128, 4, D], BF16, kind="ExternalInput")
    bqk_d = nc.dram_tensor("bqk", [128, 8], F32, kind="ExternalInput")
    bv_d = nc.dram_tensor("bv", [1, DPC], F32, kind="ExternalInput")
    mask_d = nc.dram_tensor("trimask", [128, 256], BF16, kind="ExternalInput")
    out_d = nc.dram_tensor("out", [T, D], F32, kind="ExternalOutput")
    rsum_d = nc.dram_tensor("rsum", [16, 1024], F32)  # internal scratch
    rrec_d = nc.dram_tensor("rrec", [16, 1024], F32)  # internal scratch

    with TileContext(nc) as tc:
        with (
            tc.tile_pool(name="const", bufs=1) as constp,
            tc.tile_pool(name="wstream", bufs=4) as wp,
            tc.tile_pool(name="xt", bufs=2) as xtp,
            tc.tile_pool(name="qk", bufs=1) as qkp,
            tc.tile_pool(name="vaug", bufs=1) as vp,
            tc.tile_pool(name="onorm", bufs=1) as onp,
            tc.tile_pool(name="pt", bufs=6) as ptp,
            tc.tile_pool(name="ocp", bufs=6) as ocpp,
            tc.tile_pool(name="small", bufs=8) as smallp,
            tc.tile_pool(name="osb", bufs=3) as osbp,
            tc.tile_pool(name="ps", bufs=4, space="PSUM") as psp,
            tc.tile_pool(name="spair", bufs=2, space="PSUM") as spp,
        ):
            # ---- persistent tiles ----
            qk_sb = [
                qkp.tile([128, T], BF16, tag=f"qk{j}", name=f"qk{j}")
                for j in range(8)
            ]
            vaug = [
                vp.tile([128, HPC, DH + 1], BF16, tag=f"v{t}", name=f"v{t}")
                for t in range(16)
            ]
            onorm = [
                onp.tile([128, T], BF16, tag=f"on{hp}", name=f"on{hp}")
                for hp in range(4)
            ]
            xts = [None, None]

            # ---- projection / oproj group emitters ----
            def x_load(th):
                set_label(f"xload{th}")
                xt = xtp.tile([128, 8, T // 2], BF16, tag="xt", name=f"xt{th}")
                t0 = th * (T // 2)
                for dt in range(8):
                    nc.sync.dma_start(xt[:, dt, :], xT_d[:, dt, t0 : t0 + T // 2])
                xts[th] = xt

            def j_prep(th, j):
                set_label(f"wload")
                w = wp.tile([128, 8, 128], BF16, tag="wqk")
                nc.sync.dma_start(w[:], wqk_d[:, j, :, :])
                return w

            def j_exec(th, j, tc_, w):
                set_label(f"qkv{th}")
                xt = xts[th]
                ps = psp.tile([128, 512], F32, tag="ps")
                for dt in range(8):
                    nc.tensor.matmul(
                        ps[:],
                        w[:, dt, :],
                        xt[:, dt, 512 * tc_ : 512 * (tc_ + 1)],
                        start=(dt == 0),
                        stop=(dt == 7),
                    )
                nc.vector.tensor_scalar_add(
                    qk_sb[j][
                        :, 1024 * th + 512 * tc_ : 1024 * th + 512 * (tc_ + 1)
                    ],
                    ps[:],
                    bqk_sb[:, j : j + 1],
                )

            def v_exec(th, tt):
                set_label(f"vproj{th}")
                xt = xts[th]
                tg = th * 8 + tt
                ps = psp.tile([128, 512], F32, tag="ps")
                for dt in range(8):
                    nc.tensor.matmul(
                        ps[:],
                        xt[:, dt, 128 * tt : 128 * (tt + 1)],
                        wv_sb[:, dt, :],
                        start=(dt == 0),
                        stop=(dt == 7),
                    )
                nc.vector.tensor_tensor(
                    vaug[tg][:, :, 0:DH],
                    ps[:].rearrange("p (h d) -> p h d", h=HPC),
                    bvb_sb[:].rearrange("p (h d) -> p h d", h=HPC),
                    mybir.AluOpType.add,
                )
                nc.gpsimd.memset(vaug[tg][:, :, DH : DH + 1], 1.0)

            def oproj_exec(c, g):
                set_label("oproj")
                qt = 4 * c + g // 2
                dc = g % 2
                ps = psp.tile([128, 512], F32, tag="ps")
                for hp in range(4):
                    nc.tensor.matmul(
                        ps[:],
                        onorm[hp][:, 128 * qt : 128 * (qt + 1)],
                        wo_sb[:, hp, 512 * dc : 512 * (dc + 1)],
                        start=(hp == 0),
                        stop=(hp == 3),
                    )
                osb = osbp.tile([128, 512], F32, tag="osb")
                nc.vector.tensor_copy(out=osb[:], in_=ps[:])
                nc.sync.dma_start(
                    out_d[128 * qt : 128 * (qt + 1), 512 * dc : 512 * (dc + 1)],
                    osb[:],
                )

            # filler plumbing: each filler is (prep_fn -> state, exec_fn)
            def J(th, j, tc_):
                return (lambda: j_prep(th, j), lambda w: j_exec(th, j, tc_, w))

            def V(th, tt):
                return (lambda: None, lambda _: v_exec(th, tt))

            def OP(c, g):
                return (lambda: None, lambda _: oproj_exec(c, g))

            # ---- attention chunk as a slot generator ----
            pending_norms = []

            def attn_slots(hp, c):
                qT = qk_sb[hp]
                kT = qk_sb[4 + hp]
                q0 = QC * c
                ktiles = 4 * (c + 1)
                set_label("attn")
                oA = psp.tile([128, 512], F32, tag="ps", name=f"oA{hp}_{c}")
                oB = psp.tile([128, 512], F32, tag="ps", name=f"oB{hp}_{c}")
                pts = {}

                def scores_exp(t):
                    set_label("attn")
                    j = t - 4 * c
                    qs = 128 * j if j >= 0 else 0
                    sp = spp.tile([128, 1024], F32, tag="sp")
                    for half, base in ((0, 0), (1, 64)):
                        nc.tensor.matmul(
                            sp[:, 512 * half + qs : 512 * (half + 1)],
                            kT[base : base + 64, 128 * t : 128 * (t + 1)],
                            qT[base : base + 64, q0 + qs : q0 + QC],
                            start=True,
                            stop=True,
                            tile_position=(base, 0),
                        )
                    pt = ptp.tile([128, 1024], BF16, tag="pt")
                    pts[t] = (pt, qs)
                    set_label("exp")
                    if qs == 0:
                        if use_dve_exp(hp, c, t):
                            nc.vector.tensor_scalar(
                                pt[:].bitcast(I16),
                                sp[:],
                                A_EXP,
                                B_EXP,
                                op0=mybir.AluOpType.mult,
                                op1=mybir.AluOpType.add,
                            )
                        else:
                            nc.scalar.activation(pt[:], sp[:], EXP)
                    else:
                        spv = sp[:].rearrange("p (h q) -> p h q", h=2)[:, :, qs:512]
                        ptv = pt[:].rearrange("p (h q) -> p h q", h=2)[:, :, qs:512]
                        nc.scalar.activation(ptv, spv, EXP)
                    if j >= 0:
                        ptv = pt[:].rearrange("p (h q) -> p h q", h=2)[
                            :, :, qs : qs + 128
                        ]
                        nc.gpsimd.tensor_tensor(
                            ptv, ptv, mask_sb[:], mybir.AluOpType.mult
                        )

                def av(t):
                    set_label("attn")
                    pt, qs = pts.pop(t)
                    for o_ps, half in ((oA, 0), (oB, 1)):
                        nc.tensor.matmul(
                            o_ps[0 : DH + 1, qs:512],
                            vaug[t][:, 2 * hp + half, :],
                            pt[:, 512 * half + qs : 512 * (half + 1)],
                            start=(t == 0),
                            stop=(t == ktiles - 1),
                            skip_group_check=True,
                        )

                scores_exp(0)
                yield
                for t in range(1, ktiles):
                    scores_exp(t)
                    av(t - 1)
                    yield
                av(ktiles - 1)

                # ---- normalize: drain psum early, reshape-reciprocal ----
                set_label("norm")
                ridx = hp * 4 + c
                ocps = []
                for o_ps in (oA, oB):
                    ocp = ocpp.tile([DH + 1, 512], F32, tag="ocp")
                    nc.vector.tensor_copy(out=ocp[:], in_=o_ps[0 : DH + 1, :])
                    ocps.append(ocp)
                for half, ocp in enumerate(ocps):
                    nc.sync.dma_start(
                        rsum_d[ridx : ridx + 1, 512 * half : 512 * (half + 1)],
                        ocp[DH : DH + 1, :],
                    )
                r8 = smallp.tile([128, 8], F32, tag="r8")
                nc.sync.dma_start(
                    r8[:], bass.AP(rsum_d, ridx * 1024, [[8, 128], [1, 8]])
                )
                rc8 = smallp.tile([128, 8], F32, tag="rc8")
                nc.vector.reciprocal(rc8[:], r8[:])
                nc.sync.dma_start(
                    bass.AP(rrec_d, ridx * 1024, [[8, 128], [1, 8]]), rc8[:]
                )

                # Phase 2 (broadcast + multiply) is DEFERRED into the next
                # chunk's slots so the in-order DVE stream never blocks on
                # the DMA round-trip latency.
                def finish_norm():
                    set_label("norm")
                    for half, ocp in enumerate(ocps):
                        bc = smallp.tile([64, 512], F32, tag="bc")
                        nc.sync.dma_start(
                            bc[:],
                            bass.AP(
                                rrec_d,
                                ridx * 1024 + 512 * half,
                                [[0, 64], [1, 512]],
                            ),
                        )
                        nc.vector.tensor_tensor(
                            onorm[hp][64 * half : 64 * half + 64, q0 : q0 + QC],
                            ocp[0:DH, :],
                            bc[:],
                            mybir.AluOpType.mult,
                        )

                pending_norms.append(finish_norm)
                yield

            # ---- region scheduler: interleave fillers between slots ----
            def region(chunks, fillers):
                """fillers: list of (prep, exec) or (prep, exec, deadline).
                A filler with deadline d is guaranteed to execute before the
                region's global slot index d is emitted; the rest are paced
                evenly.  Emission order IS execution order per engine, so
                deadlines encode read-after-write requirements."""
                fillers = [f if len(f) == 3 else (*f, None) for f in fillers]
                states = []

                def prep_to(i):
                    while len(states) < min(i, len(fillers)):
                        states.append(fillers[len(states)][0]())

                n_slots = sum(4 * (c + 1) + 1 for hp, c in chunks)
                prep_to(2)
                executed = 0
                slot_i = 0

                def run_due():
                    nonlocal executed
                    while executed < len(fillers):
                        dl = fillers[executed][2]
                        pace = (len(fillers) * (slot_i + 1)) // n_slots
                        if (dl is not None and dl <= slot_i) or executed < pace:
                            prep_to(executed + 3)
                            fillers[executed][1](states[executed])
                            executed += 1
                        else:
                            break

                for hp, c in chunks:
                    g = attn_slots(hp, c)
                    in_chunk = 0
                    while True:
                        run_due()
                        if in_chunk == 4:
                            while pending_norms:
                                pending_norms.pop(0)()
                        try:
                            next(g)
                        except StopIteration:
                            break
                        slot_i += 1
                        in_chunk += 1
                while executed < len(fillers):
                    prep_to(executed + 3)
                    fillers[executed][1](states[executed])
                    executed += 1
                while pending_norms:
                    pending_norms.pop(0)()

            # ---- schedule ----
            set_label("const")
            w00 = j_prep(0, 0)
            bqk_sb = constp.tile([128, 8], F32, tag="bqk")
            nc.sync.dma_start(bqk_sb[:], bqk_d[:])
            x_load(0)
            w04 = j_prep(0, 4)
            wv_sb = constp.tile([128, 8, DPC], BF16, tag="wv")
            nc.sync.dma_start(wv_sb[:], wv_d[:])
            bvb_sb = constp.tile([128, DPC], F32, tag="bvb")
            nc.sync.dma_start(bvb_sb[:], bass.AP(bv_d, 0, [[0, 128], [1, DPC]]))
            mask_sb = constp.tile([128, 2, 128], BF16, tag="mask")
            nc.sync.dma_start(
                mask_sb[:], mask_d[:].rearrange("p (h q) -> p h q", h=2)
            )

            # head: unlock (0, 0) only: Q/K tc0 j0,j4 + V tile 0
            j_exec(0, 0, 0, w00)
            j_exec(0, 4, 0, w04)
            v_exec(0, 0)

            wo_sb = constp.tile([128, 4, D], BF16, tag="wo")
            nc.sync.dma_start(wo_sb[:], wo_d[:])

            # chunk slot spans: c0 6/chunk, c1 10, c2 14, c3 18
            region(
                [(hp, 0) for hp in range(4)],
                [
                    (*V(0, 1), 1), (*V(0, 2), 2), (*V(0, 3), 3),
                    (*J(0, 1, 0), 4), (*J(0, 5, 0), 4), J(0, 0, 1), J(0, 4, 1),
                    (*J(0, 2, 0), 9), (*J(0, 6, 0), 9), J(0, 1, 1), J(0, 5, 1),
                    (*J(0, 3, 0), 14), (*J(0, 7, 0), 14), V(0, 4), V(0, 5),
                ],
            )
            x_load(1)
            region(
                [(hp, 1) for hp in range(4)],
                [
                    (*V(0, 6), 6), (*V(0, 7), 7), J(1, 0, 0), J(1, 4, 0),
                    (*J(0, 2, 1), 16), (*J(0, 6, 1), 16), OP(0, 0), OP(0, 1),
                    J(1, 1, 0), J(1, 5, 0),
                    (*J(0, 3, 1), 25), (*J(0, 7, 1), 25), OP(0, 2), OP(0, 3),
                    J(1, 2, 0), J(1, 6, 0), J(1, 3, 0), J(1, 7, 0),
                    OP(0, 4), OP(0, 5),
                ],
            )
            region(
                [(hp, 2) for hp in range(4)],
                [
                    (*V(1, 0), 7), (*V(1, 1), 8), (*V(1, 2), 9), (*V(1, 3), 10),
                    OP(0, 6), OP(0, 7), J(1, 0, 1), J(1, 4, 1),
                    OP(1, 0), OP(1, 1), J(1, 1, 1), J(1, 5, 1),
                    OP(1, 2), OP(1, 3), J(1, 2, 1), J(1, 6, 1),
                ],
            )
            region(
                [(hp, 3) for hp in range(4)],
                [
                    (*V(1, 4), 11), (*V(1, 5), 12), (*V(1, 6), 13),
                    (*V(1, 7), 14), (*J(1, 3, 1), 48), (*J(1, 7, 1), 48),
                    OP(1, 4), OP(1, 5), OP(1, 6), OP(1, 7),
                    OP(2, 0), OP(2, 1), OP(2, 2), OP(2, 3),
                ],
            )
            # tail: oproj chunk-2 remainder overlaps the final norm's DRAM
            # round trip, then chunk-3 oproj with the head-pair-3 matmuls
            # batched last so hp0-2 accumulation also runs inside that window.
            set_label("oproj")
            for g in range(4, 8):
                oproj_exec(2, g)
            for gg in (0, 4):
                groups = []
                for g in range(gg, gg + 4):
                    qt = 12 + g // 2
                    dc = g % 2
                    ps = psp.tile([128, 512], F32, tag="ps")
                    groups.append((qt, dc, ps))
                    for hp in range(3):
                        nc.tensor.matmul(
                            ps[:],
                            onorm[hp][:, 128 * qt : 128 * (qt + 1)],
                            wo_sb[:, hp, 512 * dc : 512 * (dc + 1)],
                            start=(hp == 0),
                            stop=False,
                            skip_group_check=True,
                        )
                for qt, dc, ps in groups:
                    nc.tensor.matmul(
                        ps[:],
                        onorm[3][:, 128 * qt : 128 * (qt + 1)],
                        wo_sb[:, 3, 512 * dc : 512 * (dc + 1)],
                        start=False,
                        stop=True,
                        skip_group_check=True,
                    )
                    osb = osbp.tile([128, 512], F32, tag="osb")
                    nc.vector.tensor_copy(out=osb[:], in_=ps[:])
                    nc.sync.dma_start(
                        out_d[128 * qt : 128 * (qt + 1), 512 * dc : 512 * (dc + 1)],
                        osb[:],
                    )

    split_excess_waits(nc)
    return nc


TRACE = False
LAST_EXEC_NS = None

_NC = None


def _get_nc():
    global _NC
    if _NC is None:
        _NC = build()
    return _NC


def kernel(x, W_qkv, b_qkv, W_o, b_o):
    x = np.asarray(x, dtype=np.float32)
    W_qkv = np.asarray(W_qkv, dtype=np.float32)
    b_qkv = np.asarray(b_qkv, dtype=np.float32)
    W_o = np.asarray(W_o, dtype=np.float32)
    b_o = np.asarray(b_o, dtype=np.float32)
    import ml_dtypes

    bf = ml_dtypes.bfloat16
    scale = 1.0 / np.sqrt(np.float32(DH))

    # x^T per batch, partition-major [p, dt, t] (shared between 2 cores)
    xTs = [
        np.ascontiguousarray(
            x[b].T.reshape(8, 128, T).transpose(1, 0, 2).astype(bf)
        )
        for b in range(B)
    ]

    # causal mask tile: keep iff q-local >= k-local
    tri1 = np.triu(np.ones((128, 128), np.float32))
    tri = np.concatenate([tri1, tri1], axis=1).astype(bf)

    in_maps = []
    for c in range(N_CORES):
        b, g = divmod(c, 2)
        h0 = g * HPC
        qcols = slice(h0 * DH, h0 * DH + DPC)
        kcols = slice(D + h0 * DH, D + h0 * DH + DPC)
        vcols = slice(2 * D + h0 * DH, 2 * D + h0 * DH + DPC)
        wqk = np.concatenate(
            [W_qkv[:, qcols] * scale, W_qkv[:, kcols]], axis=1
        )  # [D, 1024]
        wqk_p = wqk.reshape(8, 128, 8, 128).transpose(1, 2, 0, 3).astype(bf)
        wv_p = (
            W_qkv[:, vcols].reshape(8, 128, DPC).transpose(1, 0, 2).astype(bf)
        )
        wo_p = (
            W_o[g * DPC : (g + 1) * DPC, :]
            .reshape(4, 128, D)
            .transpose(1, 0, 2)
            .astype(bf)
        )
        bqk = np.concatenate([b_qkv[qcols] * scale, b_qkv[kcols]]).astype(
            np.float32
        )
        in_maps.append(
            {
                "xT": xTs[b],
                "wqk": np.ascontiguousarray(wqk_p),
                "wv": np.ascontiguousarray(wv_p),
                "wo": np.ascontiguousarray(wo_p),
                "bqk": np.ascontiguousarray(bqk.reshape(8, 128).T),
                "bv": np.ascontiguousarray(
                    b_qkv[vcols].reshape(1, DPC).astype(np.float32)
                ),
                "trimask": tri,
            }
        )

    nc = _get_nc()
    global LAST_EXEC_NS
    res = None
    last_err = None
    for attempt in range(3):
        try:
            res = run_bass_kernel_spmd(
                nc, in_maps, list(range(N_CORES)), trace=TRACE
            )
            break
        except Exception as e:  # transient device wedge: retry
            last_err = e
            import time as _time

            _time.sleep(5)
    if res is None:
        raise last_err
    LAST_EXEC_NS = res.exec_time_ns
    globals()["_LAST_RES"] = res
    parts = [res.results[c]["out"] for c in range(N_CORES)]
    out = np.empty((B, T, D), np.float32)
    for b in range(B):
        out[b] = parts[2 * b] + parts[2 * b + 1] + b_o[None, :]
    return out
